# revision 29
# baseline (speedup 1.0000x reference)
"""Trainium2 Bass kernel for nn_BiasedConLoss (supervised-contrastive biased loss).

Math (see reference): the only O(M^2) quantity needed is the row-wise
  Q_i = sum_j exp((A_ij - c)/T),  A = X X^T (rows L2-normalized, M=8192, D=256)
Everything else is O(M*D) on host in float64.

Device (8 NeuronCores, SPMD), per core (1024 own rows, all 8192 cols):
  GEMM in fp8e4 (features pre-scaled x16, kappa=256) using DoubleRow matmuls:
  K=256 contracted per instruction at 2 fp8 rows/cycle (2x fp16).

  The exp+row-reduce of the [1024, 8192] block is split across two engines:
  - ACT share (own 4096 cols, incl. diagonal): psum tiles [128i, 1024j],
    ScalarE Exp(in/(kappa*T) - 1/T) with accum_out giving row-sum partials
    in "c=1" units (diagonal term ~= 1, matching the reference's exp(0)=1).
  - DVE share (other 4096 cols): TRANSPOSED psum tiles [128j, 512i]. DVE
    tensor_scalar computes i8 = round(S*K8 + B8) -> int8; those bytes ARE
    the fp8e5m2 encoding of ~exp((A - c_D)/T) (bitcast exp trick, c_D=-0.2722
    chosen so A in [-1, 0.45] maps into e5m2's 32-binade range with no
    negatives / no NaN). PE DoubleRow ones-matmuls then reduce over j
    (partition dim) accumulating all pairs into one [128, 1024] psum; a
    fixed calibration constant C_CAL (=1/E[decode/exp], measured 0.96209)
    removes the piecewise-linear decode bias on host.

  PSUM (8 banks): ACT 2x[128,1024] | P_T 3x[128,512] | QD [128,512].
  The two i-halves' rowsums run as two epochs sharing the one QD bank
  (evacuated between). Standalone LDWEIGHTS "fences" (one per input DMA
  group; a weight load has no psum operand so it carries exactly one wait)
  keep every later instruction at ONE sync-wait (walrus limit): post-fence
  only psum-WAR semaphores remain live.

  Numerics (rel err ~1.3e-3 vs the 2e-2 gate):
  - the 64 diagonal [128,128] blocks are computed on host in f64 (the ACT
    Exp LUT's one-sided Taylor error there is ~5% of the off-diag signal);
  - C_CAL removes the e5m2 piecewise-linear decode bias (DVE share);
  - C_Q removes the e4m3 feature-quantization bias of the off-diag sum.

  Symmetric steps: the c=1 and c=2 ACT units (local slices 2-3 / 4-5 = the
  rows of cores c+1 / c+2) write their exp values as e5m2 y tiles; DR-ones
  colsum matmuls (QC bank, 2 epochs each) produce column-partials that ARE
  the mirror sums for those cores' rows -- so every core drops local slices
  12-15 entirely (-25% exp work; slices 12-15 are not even DMA'd) and the
  freed c=3 ACT share moved to DVE (slices 6-11). C_SYM=1.0027 calibrates
  the e5m2 rounding of the colsum path.

  Measured: 55.6 us vs 89.3 us fp16 baseline (1.61x) under quiet device
  conditions (device-state variance is +-15%); ACT ~42 / DVE ~46 / PE ~38
  us busy. PE emission is ordered so P_T matmuls run ahead of ACT-unit
  matmuls (whose psum-WAR waits would otherwise head-of-line-block the DVE
  stream); warmup is a single tiny matmul; QC evacs run on the tail-idle
  ACT engine; input DMAs use 4KB contiguous per-partition descriptors.
"""
import numpy as np
import ml_dtypes

import concourse.bass as bass
import concourse.tile as tile
from concourse import mybir
from concourse.bass_utils import run_bass_kernel_spmd
from concourse.vector_clock import ScopedClock, VectorClock

F32 = mybir.dt.float32
F16 = mybir.dt.float16
F8E4 = mybir.dt.float8e4
F8E5 = mybir.dt.float8e5
I8 = mybir.dt.int8

T = 0.07
N = 4096
D = 256
M = 2 * N                      # 8192
NCORES = 8
ROWS_PER_CORE = M // NCORES    # 1024
NSLICE = 16                    # 512-col slices of the j axis
KAPPA = 256.0                  # fp8 pre-scale 16 squared
LOG2E = float(np.log2(np.e))
K8 = 4.0 * LOG2E / (T * KAPPA)
C_D = -0.2722
B8 = 4.0 * (15.0 - C_D * LOG2E / T)
C_CAL = 0.9620892974373026     # e5m2 bitcast-exp decode bias (staged-input calib)
C_Q = 0.9856599143895837       # e4m3 feature-quantization bias on off-diag Q'

NACT_CHUNK = 4                 # ACT units (1024 cols) per i-block
NPAIR = 12                     # DVE pairs (slices 8-13; 14-15 covered by core c-1's SYM colsums)
C_SYM = 1.002695117350887      # e5m2 round-to-nearest bias of the SYM y values

_SELF_SEM_PREFIX = {
    mybir.EngineType.PE: "PE_",
    mybir.EngineType.Activation: "Activation_",
    mybir.EngineType.DVE: "DVE_",
}


class _SplitDrainTileContext(tile.TileContext):
    """Walrus-compat: strip same-engine semaphore self-waits (PE/ACT/DVE are
    in-order engines, so waits on the engine's own completion semaphore are
    redundant with program order) and split the kernel-tail drain's waits
    across many Drain instructions (walrus allows ONE sync-wait per
    instruction)."""

    def _lower_ordered_insts(self, postordered_blocks):
        for insts in postordered_blocks.values():
            for inst in insts:
                si = getattr(inst, "sync_info", None)
                if si is None or not si.on_wait:
                    continue
                prefix = _SELF_SEM_PREFIX.get(inst.engine)
                kept = si.on_wait
                if prefix is not None:
                    kept = [
                        w for w in kept
                        if not (w.ant_name or "").startswith(prefix)
                    ]
                if (
                    inst.engine == mybir.EngineType.Pool
                    and type(inst).__name__ == "InstDMACopy"
                ):
                    # Pool only issues the SWDGE output stores; DMASW waits
                    # are same-queue FIFO ordering (redundant in-order).
                    kept = [
                        w for w in kept
                        if not (w.ant_name or "").startswith("DMASW")
                    ]
                if len(kept) != len(si.on_wait):
                    si.on_wait = kept
        return super()._lower_ordered_insts(postordered_blocks)

    def _drain_and_barrier(self, tick_clock, wait_clock):
        full = tick_clock.global_clock
        n = len(full)
        procs = [p for p in range(n) if full[p] > 0]
        for p in procs:
            vec = [full[q] if q == p else 0 for q in range(n)]
            d = self.nc.sync.drain()
            wait_clock.add_sem_waits(d.ins, ScopedClock({None: VectorClock(vec)}))
        if not procs:
            d = self.nc.sync.drain()
            wait_clock.add_sem_waits(
                d.ins, ScopedClock({None: tick_clock.global_clock})
            )
        self.nc.all_engine_barrier()
        assert self.sems is not None
        popped = self.nc._tile_sem_poison_stack.pop()
        assert popped is self._sem_poison
        self.nc.clear_and_free_semaphores(list(self.sems.allocated().values()))
        self.nc.all_engine_barrier()


def _build():
    nc = bass.Bass("TRN2", target_bir_lowering=False, debug=False,
                   num_swdge_queues=1)
    xe = nc.dram_tensor("xe", [4, 128, 4096], F8E4, kind="ExternalInput").ap()
    stats = nc.dram_tensor("stats", [128, 40], F32, kind="ExternalOutput").ap()
    qd_out = nc.dram_tensor("qd", [1, 1024], F32, kind="ExternalOutput").ap()
    qc_out = nc.dram_tensor("qc", [1, 1024], F32, kind="ExternalOutput").ap()
    qc2_out = nc.dram_tensor("qc2", [1, 1024], F32, kind="ExternalOutput").ap()

    xin_t = nc.alloc_sbuf_tensor("xin", [128, NSLICE, 2, 512], F8E4)
    ones_t = nc.alloc_sbuf_tensor("ones8", [128, 2, 128], F8E5)
    bias_t = nc.alloc_sbuf_tensor("bias_const", [128, 1], F32)
    warm_t = nc.alloc_sbuf_tensor("warm_zeros", [128, 512], F16)
    stats_t = nc.alloc_sbuf_tensor("stats_sb", [128, 40], F32)
    biasd_t = nc.alloc_sbuf_tensor("bias_d", [128, 1], F32)
    ysym_t = nc.alloc_sbuf_tensor("ysym", [128, 4, 2, 1024], I8)
    ysym2_t = nc.alloc_sbuf_tensor("ysym2", [128, 4, 2, 1024], I8)
    qc2_sb0_t = nc.alloc_sbuf_tensor("qc2_sb0", [1, 512], F32)
    qc2_sb1_t = nc.alloc_sbuf_tensor("qc2_sb1", [1, 512], F32)
    qc_sb0_t = nc.alloc_sbuf_tensor("qc_sb0", [1, 512], F32)
    qc_sb1_t = nc.alloc_sbuf_tensor("qc_sb1", [1, 512], F32)
    qd_sb0_t = nc.alloc_sbuf_tensor("qd_sb0", [1, 512], F32)
    qd_sb1_t = nc.alloc_sbuf_tensor("qd_sb1", [1, 512], F32)
    # y slots: 4 in rotation, each [128, 2, 512] int8 (a (pair, i-half) tile)
    y_t = nc.alloc_sbuf_tensor("y_sb", [128, 8, 2, 512], I8)

    with _SplitDrainTileContext(nc) as tc:
        ones_c = nc.const_aps.tensor(1.0, (128, 1), mybir.dt.float32)
        nc.scalar.mul(bias_t.ap(), ones_c, -1.0 / T)
        nc.scalar.mul(biasd_t.ap(), ones_c, -C_D / T)
        nc.vector.memset(ones_t.ap(), 1.0)

        xin = xin_t.ap()
        ysb = y_t.ap()
        with tc.tile_pool(name="act", bufs=2, space="PSUM") as act_pool, \
             tc.tile_pool(name="pt", bufs=2, space="PSUM") as pt_pool, \
             tc.tile_pool(name="qd", bufs=1, space="PSUM") as qd_pool, \
             tc.tile_pool(name="qc", bufs=1, space="PSUM") as qc_pool:

            # input DMAs: 4 groups of 4 slices; g2 (first DVE slices) early so
            # the DVE stream starts right after the ACT stream
            # group 3 (slices 12-15) is never read: slices 12-15 are covered
            # by other cores' SYM colsums, so skip its DMA entirely
            for g in (0, 2, 1):
                nc.sync.dma_start(
                    out=xin[:, 4 * g:4 * (g + 1), :, :],
                    in_=xe[g],
                )

            def own_lhsT(r):
                # own i-block r (128 rows): local slices 0..1, col offset
                return xin[:, r // 4, :, (r % 4) * 128:(r % 4) * 128 + 128]

            # PE warm-up (HAM clock throttle release); no input deps
            warm_ap = warm_t.ap()
            ps_warm = act_pool.tile([128, 1024], F32, tag="act")
            nc.tensor.matmul(
                ps_warm[0:1, 0:64],
                lhsT=warm_ap[:, 0:1], rhs=warm_ap[:, 0:64],
                start=True, stop=True, skip_group_check=True,
            )

            stats_ap = stats_t.ap()

            def act_unit(r, c):
                """ACT unit: i-block r, cols = local slices (2c, 2c+1).

                For c==0 (the own-column chunk) the i-block's own 128-col
                diagonal block is skipped: its exponents spread +-0.3 around
                zero where the ACT Exp LUT's piecewise-Taylor error (~-4e-4,
                one-sided) is 5%-of-signal after recentering. The host adds
                those 64 [128,128] blocks exactly in f64. Clean columns are
                packed contiguously so one activation covers them."""
                psA = act_pool.tile([128, 1024], F32, tag="act")
                lhsT = own_lhsT(r)
                if c == 0:
                    runs = [(0, 128 * r), (128 * r + 128, 1024)]
                    width = 896
                    base = 0
                else:
                    runs = [(0, 1024)]
                    width = 1024
                    base = 2 * c * 512
                dst = 0
                for a, b in runs:
                    c0 = a
                    while c0 < b:
                        c1 = min(b, (c0 // 512 + 1) * 512)
                        room = 512 - (dst % 512) if dst % 512 else 512
                        ln = min(c1 - c0, room)
                        s = (base + c0) // 512
                        o = (base + c0) % 512
                        nc.tensor.matmul(
                            psA[:, dst:dst + ln],
                            lhsT=lhsT, rhs=xin[:, s, :, o:o + ln],
                            start=True, stop=True, skip_group_check=True,
                            perf_mode=mybir.MatmulPerfMode.DoubleRow,
                        )
                        dst += ln
                        c0 += ln
                if c in (1, 2):
                    # SYM unit: exp in c_D units, y -> SBUF e5m2 for the
                    # colsum pass (mirror contributions for core c+c's rows)
                    yt = ysym_t if c == 1 else ysym2_t
                    out_ap = yt.ap()[:, r // 2, r % 2, :].bitcast(F8E5)
                    bias_ap = biasd_t.ap()
                else:
                    out_ap = psA[:, 0:width]
                    bias_ap = bias_t.ap()
                nc.scalar.activation(
                    out=out_ap, in_=psA[:, 0:width],
                    func=mybir.ActivationFunctionType.Exp,
                    scale=1.0 / (T * KAPPA), bias=bias_ap,
                    accum_out=stats_ap[:, (r * 5 + c):(r * 5 + c) + 1],
                )

            qd_ps = qd_pool.tile([128, 512], F32, tag="qd")
            slot_ctr = [0]
            count_ih = [0, 0]

            def dve_quarter(p, ih):
                """One (pair p, i-half ih): 2 transposed P_T tiles + convs +
                one DR-ones rowsum-mm accumulating into qd_ps. The i-halves
                run as two epochs sharing one QD bank (evacuated between)."""
                yslot = slot_ctr[0] % 8
                slot_ctr[0] += 1
                n_epoch = NPAIR
                first = count_ih[ih] == 0
                last = count_ih[ih] == n_epoch - 1
                count_ih[ih] += 1
                for h in range(2):
                    # j-block = local slice 6 + p//2, block (2*(p%2) + h)
                    s = 6 + p // 2
                    o = (2 * (p % 2) + h) * 128
                    pt = pt_pool.tile([128, 512], F32, tag="pt")
                    nc.tensor.matmul(
                        pt[:],
                        lhsT=xin[:, s, :, o:o + 128],
                        rhs=xin[:, ih, :, :],
                        start=True, stop=True, skip_group_check=True,
                        perf_mode=mybir.MatmulPerfMode.DoubleRow,
                    )
                    nc.vector.tensor_scalar(
                        out=ysb[:, yslot, h:h + 1, :], in0=pt[:],
                        scalar1=float(K8), scalar2=float(B8),
                        op0=mybir.AluOpType.mult, op1=mybir.AluOpType.add,
                    )
                y8 = ysb[:, yslot, :, :].bitcast(F8E5)
                nc.tensor.matmul(
                    qd_ps[:],
                    lhsT=ones_t.ap(), rhs=y8,
                    start=first, stop=last, skip_group_check=True,
                    perf_mode=mybir.MatmulPerfMode.DoubleRow,
                )

            # pre-fence: chunk-0 ACT units (DMA group 0) interleaved with the
            # first epoch-0 quarters (pairs 0-7: slices 8-11 = group 2, own
            # i-lo rhs = slice 0 = group 0)
            pre = [("act", r, 0) for r in range(8)]
            preq = [("q", p, 0) for p in range(4, 12)]
            order = [pre[0], pre[1], preq[0], preq[1], pre[2], preq[2],
                     preq[3], pre[3], preq[4], preq[5], pre[4], preq[6],
                     preq[7], pre[5], pre[6], pre[7]]
            for it in order:
                if it[0] == "act":
                    act_unit(it[1], it[2])
                else:
                    dve_quarter(it[1], it[2])

            # fence: standalone LDWEIGHTS per not-yet-consumed DMA group; a
            # pure weight load has no psum operand, so it carries exactly ONE
            # wait (that group's DMA semaphore). Later PE instructions then
            # transitively dominate all input DMAs (1-wait walrus limit).
            for s in (7,):      # group 1 (groups 0/2 are consumed pre-fence)
                nc.tensor.ldweights(
                    weights=xin[:, s, :, 0:128],
                    perf_mode=mybir.MatmulPerfMode.DoubleRow,
                )

            # main: remaining ACT units (c=1..3 all blocks, c=4 only i-hi
            # blocks) interleaved with the remaining quarters (epoch-0 pairs
            # 8-15 incl slice-15 i-lo pairs 14/15, then epoch-1 pairs 0-13;
            # the QD bank is evacuated between epochs and reused)
            units = ([(r, 1) for r in range(8)] + ["colsum"]
                     + [(r, 2) for r in range(8)] + ["colsum2"])
            quarters = ([(p, 0) for p in range(0, 4)]
                        + [None]
                        + [(p, 1) for p in range(NPAIR)])

            def colsum_section(yt, sb0, sb1, dram_out):
                # QC [128,512]: two epochs (one per 512-chunk of the SYM
                # window); 4 pair DR-ones-mms accumulate, then evac + store
                for chunk, (sbuf, dram) in enumerate(
                        [(sb0, dram_out[:, 0:512]),
                         (sb1, dram_out[:, 512:1024])]):
                    qc_ps = qc_pool.tile([128, 512], F32, tag="qc")
                    for pair in range(4):
                        y8s = yt.ap()[
                            :, pair, :, 512 * chunk:512 * (chunk + 1)
                        ].bitcast(F8E5)
                        nc.tensor.matmul(
                            qc_ps[:], lhsT=ones_t.ap(), rhs=y8s,
                            start=(pair == 0), stop=(pair == 3),
                            skip_group_check=True,
                            perf_mode=mybir.MatmulPerfMode.DoubleRow,
                        )
                    nc.scalar.activation(
                        out=sbuf.ap(), in_=qc_ps[0:1, :],
                        func=mybir.ActivationFunctionType.Copy, bias=0.0,
                    )
                    nc.gpsimd.dma_start(out=dram, in_=sbuf.ap())
            def emit_q():
                q = quarters.pop(0)
                if q is None:
                    # epoch-0 QD evac (DVE) + store; epoch-1 reuses the bank
                    nc.vector.tensor_copy(qd_sb0_t.ap(), qd_ps[0:1, :])
                    nc.gpsimd.dma_start(out=qd_out[:, 0:512],
                                        in_=qd_sb0_t.ap())
                else:
                    dve_quarter(*q)

            for _ in range(2):
                if quarters:
                    emit_q()
            for u in units:
                if u == "colsum":
                    colsum_section(ysym_t, qc_sb0_t, qc_sb1_t, qc_out)
                    continue
                if u == "colsum2":
                    colsum_section(ysym2_t, qc2_sb0_t, qc2_sb1_t, qc2_out)
                    continue
                act_unit(*u)
                if quarters:
                    emit_q()
            while quarters:
                emit_q()

            # epoch-1 QD evac; outputs split so each store DMA has one wait
            nc.vector.tensor_copy(qd_sb1_t.ap(), qd_ps[0:1, :])
            nc.gpsimd.dma_start(out=stats, in_=stats_t.ap())
            nc.gpsimd.dma_start(out=qd_out[:, 512:1024],
                                in_=qd_sb1_t.ap())
    return nc


_NC_CACHE = None


def _get_nc():
    global _NC_CACHE
    if _NC_CACHE is None:
        _NC_CACHE = _build()
    return _NC_CACHE


def kernel(labels, all_features, all_features_cr, _trace=False):
    labels = np.asarray(labels)
    f = np.asarray(all_features, dtype=np.float32)
    f_cr = np.asarray(all_features_cr, dtype=np.float32)

    X = np.concatenate([f, f_cr], axis=0)                 # [M, D] f32
    X8 = (X * 16.0).astype(ml_dtypes.float8_e4m3)         # device quantization
    XT8 = np.ascontiguousarray(X8.T)                      # [D, M]

    in_maps = []
    for c in range(NCORES):
        xe = np.empty((4, 128, 4096), dtype=ml_dtypes.float8_e4m3)
        for s in range(NSLICE):
            gs = (2 * c + s) % NSLICE
            g, o = s // 4, (s % 4) * 1024
            xe[g, :, o:o + 512] = XT8[0:128, 512 * gs:512 * (gs + 1)]
            xe[g, :, o + 512:o + 1024] = XT8[128:256, 512 * gs:512 * (gs + 1)]
        in_maps.append({"xe": xe})

    nc = _get_nc()
    res = run_bass_kernel_spmd(
        nc, in_maps, core_ids=list(range(NCORES)), trace=_trace
    )
    kernel.last_exec_time_ns = res.exec_time_ns
    kernel.last_trace = res.instructions_and_trace
    kernel.last_results = res.results

    # ---- host epilogue (float64, O(M*D)) ----
    X8f = X8.astype(np.float64) / 16.0                    # device-seen features
    d_hat = np.sum(X8f * X8f, axis=1)                     # device diag of A

    Q1 = np.empty(M, dtype=np.float64)                    # c=1 units, incl diag
    e_shift = C_CAL * np.exp((C_D - 1.0) / T)
    e_shift_sym = np.exp((C_D - 1.0) / T)
    for core in range(NCORES):
        st = res.results[core]["stats"].astype(np.float64)    # [128, 32]
        qd = res.results[core]["qd"].astype(np.float64)[0]    # [1024]
        g0 = core * ROWS_PER_CORE
        for r in range(8):
            cols = st[:, 5 * r:5 * r + 4]
            acc = cols[:, 0]                                 # c_A=1 unit (c0)
            acc_sym = (cols[:, 1] + cols[:, 2]) * e_shift_sym  # SYM units
            i0 = g0 + 128 * r
            Q1[i0:i0 + 128] = (acc + acc_sym
                               + e_shift * qd[128 * r:128 * (r + 1)])

    # mirror contributions: core c's SYM colsums are the (dropped) slices
    # 14-15 sums for core (c+1)'s rows
    for core in range(NCORES):
        qc = res.results[core]["qc"].astype(np.float64)[0]    # [1024]
        qc2 = res.results[core]["qc2"].astype(np.float64)[0]
        g1 = ((core + 1) % NCORES) * ROWS_PER_CORE
        g2 = ((core + 2) % NCORES) * ROWS_PER_CORE
        Q1[g1:g1 + 1024] += C_SYM * e_shift_sym * qc
        Q1[g2:g2 + 1024] += C_SYM * e_shift_sym * qc2

    # diagonal 128-blocks, exactly in f64 (the device skips them)
    for b in range(M // 128):
        blk = X8f[128 * b:128 * (b + 1)]
        Sb = blk @ blk.T
        Q1[128 * b:128 * (b + 1)] += np.exp((Sb - 1.0) / T).sum(axis=1)

    # diag term recenters to exactly 1 (matching the reference's exp(0));
    # C_Q removes the systematic e4m3-quantization bias of the off-diag sum
    row_sum = 1.0 + C_Q * (Q1 * np.exp((1.0 - d_hat) / T) - 1.0)
    row_logsum = np.log(row_sum)

    Xd = X.astype(np.float64)
    lab = np.asarray(labels)
    all_labels = np.concatenate([lab, lab]).astype(np.float64)
    pos_f = (all_labels == 1).astype(np.float64)
    neg_f = 1.0 - pos_f
    P = pos_f.sum()
    U = neg_f.sum()

    d_true = np.sum(Xd * Xd, axis=1)
    w_pos = pos_f @ Xd
    pos_dot_raw = Xd @ w_pos
    spos = (pos_dot_raw - P * d_true) / T
    sup_row = spos - M * row_logsum
    loss_sup = np.sum(pos_f * (-sup_row / P)) / P

    partner = np.sum(Xd * np.roll(Xd, -N, axis=0), axis=1)
    unsup_row = (partner - d_true) / T - M * row_logsum
    loss_unsup = np.sum(neg_f * (-unsup_row / U)) / U

    return (np.float32(loss_sup), np.float32(loss_unsup))


# revision 30
# speedup vs baseline: 1.0480x; 1.0480x over previous
"""Trainium2 Bass kernel for nn_BiasedConLoss (supervised-contrastive biased loss).

Math (see reference): the only O(M^2) quantity needed is the row-wise
  Q_i = sum_j exp((A_ij - c)/T),  A = X X^T (rows L2-normalized, M=8192, D=256)
Everything else is O(M*D) on host in float64.

Device (8 NeuronCores, SPMD), per core (1024 own rows, all 8192 cols):
  GEMM in fp8e4 (features pre-scaled x16, kappa=256) using DoubleRow matmuls:
  K=256 contracted per instruction at 2 fp8 rows/cycle (2x fp16).

  The exp+row-reduce of the [1024, 8192] block is split across two engines:
  - ACT share (own 4096 cols, incl. diagonal): psum tiles [128i, 1024j],
    ScalarE Exp(in/(kappa*T) - 1/T) with accum_out giving row-sum partials
    in "c=1" units (diagonal term ~= 1, matching the reference's exp(0)=1).
  - DVE share (other 4096 cols): TRANSPOSED psum tiles [128j, 512i]. DVE
    tensor_scalar computes i8 = round(S*K8 + B8) -> int8; those bytes ARE
    the fp8e5m2 encoding of ~exp((A - c_D)/T) (bitcast exp trick, c_D=-0.2722
    chosen so A in [-1, 0.45] maps into e5m2's 32-binade range with no
    negatives / no NaN). PE DoubleRow ones-matmuls then reduce over j
    (partition dim) accumulating all pairs into one [128, 1024] psum; a
    fixed calibration constant C_CAL (=1/E[decode/exp], measured 0.96209)
    removes the piecewise-linear decode bias on host.

  PSUM (8 banks): ACT 2x[128,1024] | P_T 3x[128,512] | QD [128,512].
  The two i-halves' rowsums run as two epochs sharing the one QD bank
  (evacuated between). Standalone LDWEIGHTS "fences" (one per input DMA
  group; a weight load has no psum operand so it carries exactly one wait)
  keep every later instruction at ONE sync-wait (walrus limit): post-fence
  only psum-WAR semaphores remain live.

  Numerics (rel err ~1.3e-3 vs the 2e-2 gate):
  - the 64 diagonal [128,128] blocks are computed on host in f64 (the ACT
    Exp LUT's one-sided Taylor error there is ~5% of the off-diag signal);
  - C_CAL removes the e5m2 piecewise-linear decode bias (DVE share);
  - C_Q removes the e4m3 feature-quantization bias of the off-diag sum.

  Symmetric steps: the c=1 and c=2 ACT units (local slices 2-3 / 4-5 = the
  rows of cores c+1 / c+2) write their exp values as e5m2 y tiles; DR-ones
  colsum matmuls (QC bank, 2 epochs each) produce column-partials that ARE
  the mirror sums for those cores' rows -- so every core drops local slices
  12-15 entirely (-25% exp work; slices 12-15 are not even DMA'd) and the
  freed c=3 ACT share moved to DVE (slices 6-11). C_SYM=1.0027 calibrates
  the e5m2 rounding of the colsum path.

  Measured: 55.6 us vs 89.3 us fp16 baseline (1.61x) under quiet device
  conditions (device-state variance is +-15%); ACT ~42 / DVE ~46 / PE ~38
  us busy. PE emission is ordered so P_T matmuls run ahead of ACT-unit
  matmuls (whose psum-WAR waits would otherwise head-of-line-block the DVE
  stream); warmup is a single tiny matmul; QC evacs run on the tail-idle
  ACT engine; input DMAs use 4KB contiguous per-partition descriptors.
"""
import numpy as np
import ml_dtypes

import concourse.bass as bass
import concourse.tile as tile
from concourse import mybir
from concourse.bass_utils import run_bass_kernel_spmd
from concourse.vector_clock import ScopedClock, VectorClock

F32 = mybir.dt.float32
F16 = mybir.dt.float16
F8E4 = mybir.dt.float8e4
F8E5 = mybir.dt.float8e5
I8 = mybir.dt.int8

T = 0.07
N = 4096
D = 256
M = 2 * N                      # 8192
NCORES = 8
ROWS_PER_CORE = M // NCORES    # 1024
NSLICE = 16                    # 512-col slices of the j axis
KAPPA = 256.0                  # fp8 pre-scale 16 squared
LOG2E = float(np.log2(np.e))
K8 = 4.0 * LOG2E / (T * KAPPA)
C_D = -0.2722
B8 = 4.0 * (15.0 - C_D * LOG2E / T)
C_CAL = 0.9620892974373026     # e5m2 bitcast-exp decode bias (staged-input calib)
C_Q = 0.9856599143895837       # e4m3 feature-quantization bias on off-diag Q'

NACT_CHUNK = 4                 # ACT units (1024 cols) per i-block
NPAIR = 12                     # DVE pairs (slices 8-13; 14-15 covered by core c-1's SYM colsums)
C_SYM = 1.002695117350887      # e5m2 round-to-nearest bias of the SYM y values

_SELF_SEM_PREFIX = {
    mybir.EngineType.PE: "PE_",
    mybir.EngineType.Activation: "Activation_",
    mybir.EngineType.DVE: "DVE_",
}


class _SplitDrainTileContext(tile.TileContext):
    """Walrus-compat: strip same-engine semaphore self-waits (PE/ACT/DVE are
    in-order engines, so waits on the engine's own completion semaphore are
    redundant with program order) and split the kernel-tail drain's waits
    across many Drain instructions (walrus allows ONE sync-wait per
    instruction)."""

    def _lower_ordered_insts(self, postordered_blocks):
        for insts in postordered_blocks.values():
            for inst in insts:
                si = getattr(inst, "sync_info", None)
                if si is None or not si.on_wait:
                    continue
                prefix = _SELF_SEM_PREFIX.get(inst.engine)
                kept = si.on_wait
                if prefix is not None:
                    kept = [
                        w for w in kept
                        if not (w.ant_name or "").startswith(prefix)
                    ]
                if (
                    inst.engine == mybir.EngineType.Pool
                    and type(inst).__name__ == "InstDMACopy"
                ):
                    # Pool only issues the SWDGE output stores; DMASW waits
                    # are same-queue FIFO ordering (redundant in-order).
                    kept = [
                        w for w in kept
                        if not (w.ant_name or "").startswith("DMASW")
                    ]
                if len(kept) != len(si.on_wait):
                    si.on_wait = kept
        return super()._lower_ordered_insts(postordered_blocks)

    def _drain_and_barrier(self, tick_clock, wait_clock):
        full = tick_clock.global_clock
        n = len(full)
        procs = [p for p in range(n) if full[p] > 0]
        for p in procs:
            vec = [full[q] if q == p else 0 for q in range(n)]
            d = self.nc.sync.drain()
            wait_clock.add_sem_waits(d.ins, ScopedClock({None: VectorClock(vec)}))
        if not procs:
            d = self.nc.sync.drain()
            wait_clock.add_sem_waits(
                d.ins, ScopedClock({None: tick_clock.global_clock})
            )
        self.nc.all_engine_barrier()
        assert self.sems is not None
        popped = self.nc._tile_sem_poison_stack.pop()
        assert popped is self._sem_poison
        self.nc.clear_and_free_semaphores(list(self.sems.allocated().values()))
        self.nc.all_engine_barrier()


def _build():
    nc = bass.Bass("TRN2", target_bir_lowering=False, debug=False,
                   num_swdge_queues=1)
    xe = nc.dram_tensor("xe", [4, 128, 4096], F8E4, kind="ExternalInput").ap()
    stats = nc.dram_tensor("stats", [128, 40], F32, kind="ExternalOutput").ap()
    qd_out = nc.dram_tensor("qd", [1, 1024], F32, kind="ExternalOutput").ap()
    qc_out = nc.dram_tensor("qc", [1, 1024], F32, kind="ExternalOutput").ap()
    qc2_out = nc.dram_tensor("qc2", [1, 1024], F32, kind="ExternalOutput").ap()

    xin_t = nc.alloc_sbuf_tensor("xin", [128, NSLICE, 2, 512], F8E4)
    ones_t = nc.alloc_sbuf_tensor("ones8", [128, 2, 128], F8E5)
    bias_t = nc.alloc_sbuf_tensor("bias_const", [128, 1], F32)
    warm_t = nc.alloc_sbuf_tensor("warm_zeros", [128, 512], F16)
    stats_t = nc.alloc_sbuf_tensor("stats_sb", [128, 40], F32)
    biasd_t = nc.alloc_sbuf_tensor("bias_d", [128, 1], F32)
    ysym_t = nc.alloc_sbuf_tensor("ysym", [128, 4, 2, 1024], I8)
    ysym2_t = nc.alloc_sbuf_tensor("ysym2", [128, 4, 2, 1024], I8)
    qc2_sb0_t = nc.alloc_sbuf_tensor("qc2_sb0", [1, 512], F32)
    qc2_sb1_t = nc.alloc_sbuf_tensor("qc2_sb1", [1, 512], F32)
    qc_sb0_t = nc.alloc_sbuf_tensor("qc_sb0", [1, 512], F32)
    qc_sb1_t = nc.alloc_sbuf_tensor("qc_sb1", [1, 512], F32)
    qd_sb0_t = nc.alloc_sbuf_tensor("qd_sb0", [1, 512], F32)
    qd_sb1_t = nc.alloc_sbuf_tensor("qd_sb1", [1, 512], F32)
    # y slots: 4 in rotation, each [128, 2, 512] int8 (a (pair, i-half) tile)
    y_t = nc.alloc_sbuf_tensor("y_sb", [128, 8, 2, 512], I8)

    with _SplitDrainTileContext(nc) as tc:
        ones_c = nc.const_aps.tensor(1.0, (128, 1), mybir.dt.float32)
        nc.scalar.mul(bias_t.ap(), ones_c, -1.0 / T)
        nc.scalar.mul(biasd_t.ap(), ones_c, -C_D / T)
        nc.vector.memset(ones_t.ap(), 1.0)

        xin = xin_t.ap()
        ysb = y_t.ap()
        with tc.tile_pool(name="act", bufs=2, space="PSUM") as act_pool, \
             tc.tile_pool(name="pt", bufs=3, space="PSUM") as pt_pool, \
             tc.tile_pool(name="qd", bufs=1, space="PSUM") as qd_pool:

            # input DMAs: 4 groups of 4 slices; g2 (first DVE slices) early so
            # the DVE stream starts right after the ACT stream
            # group 3 (slices 12-15) is never read: slices 12-15 are covered
            # by other cores' SYM colsums, so skip its DMA entirely
            for g in (0, 2, 1):
                nc.sync.dma_start(
                    out=xin[:, 4 * g:4 * (g + 1), :, :],
                    in_=xe[g],
                )

            def own_lhsT(r):
                # own i-block r (128 rows): local slices 0..1, col offset
                return xin[:, r // 4, :, (r % 4) * 128:(r % 4) * 128 + 128]

            # PE warm-up (HAM clock throttle release); no input deps
            warm_ap = warm_t.ap()
            ps_warm = act_pool.tile([128, 1024], F32, tag="act")
            nc.tensor.matmul(
                ps_warm[0:1, 0:64],
                lhsT=warm_ap[:, 0:1], rhs=warm_ap[:, 0:64],
                start=True, stop=True, skip_group_check=True,
            )

            stats_ap = stats_t.ap()

            def act_unit(r, c):
                """ACT unit: i-block r, cols = local slices (2c, 2c+1).

                For c==0 (the own-column chunk) the i-block's own 128-col
                diagonal block is skipped: its exponents spread +-0.3 around
                zero where the ACT Exp LUT's piecewise-Taylor error (~-4e-4,
                one-sided) is 5%-of-signal after recentering. The host adds
                those 64 [128,128] blocks exactly in f64. Clean columns are
                packed contiguously so one activation covers them."""
                psA = act_pool.tile([128, 1024], F32, tag="act")
                lhsT = own_lhsT(r)
                if c == 0:
                    runs = [(0, 128 * r), (128 * r + 128, 1024)]
                    width = 896
                    base = 0
                else:
                    runs = [(0, 1024)]
                    width = 1024
                    base = 2 * c * 512
                dst = 0
                for a, b in runs:
                    c0 = a
                    while c0 < b:
                        c1 = min(b, (c0 // 512 + 1) * 512)
                        room = 512 - (dst % 512) if dst % 512 else 512
                        ln = min(c1 - c0, room)
                        s = (base + c0) // 512
                        o = (base + c0) % 512
                        nc.tensor.matmul(
                            psA[:, dst:dst + ln],
                            lhsT=lhsT, rhs=xin[:, s, :, o:o + ln],
                            start=True, stop=True, skip_group_check=True,
                            perf_mode=mybir.MatmulPerfMode.DoubleRow,
                        )
                        dst += ln
                        c0 += ln
                if c in (1, 2):
                    # SYM unit: exp in c_D units, y -> SBUF e5m2 for the
                    # colsum pass (mirror contributions for core c+c's rows)
                    yt = ysym_t if c == 1 else ysym2_t
                    out_ap = yt.ap()[:, r // 2, r % 2, :].bitcast(F8E5)
                    bias_ap = biasd_t.ap()
                else:
                    out_ap = psA[:, 0:width]
                    bias_ap = bias_t.ap()
                nc.scalar.activation(
                    out=out_ap, in_=psA[:, 0:width],
                    func=mybir.ActivationFunctionType.Exp,
                    scale=1.0 / (T * KAPPA), bias=bias_ap,
                    accum_out=stats_ap[:, (r * 5 + c):(r * 5 + c) + 1],
                )

            qd_ps = qd_pool.tile([128, 512], F32, tag="qd")
            slot_ctr = [0]
            count_ih = [0, 0]

            def dve_quarter(p, ih):
                """One (pair p, i-half ih): 2 transposed P_T tiles + convs +
                one DR-ones rowsum-mm accumulating into qd_ps. The i-halves
                run as two epochs sharing one QD bank (evacuated between)."""
                yslot = slot_ctr[0] % 8
                slot_ctr[0] += 1
                n_epoch = NPAIR
                first = count_ih[ih] == 0
                last = count_ih[ih] == n_epoch - 1
                count_ih[ih] += 1
                for h in range(2):
                    # j-block = local slice 6 + p//2, block (2*(p%2) + h)
                    s = 6 + p // 2
                    o = (2 * (p % 2) + h) * 128
                    pt = pt_pool.tile([128, 512], F32, tag="pt")
                    nc.tensor.matmul(
                        pt[:],
                        lhsT=xin[:, s, :, o:o + 128],
                        rhs=xin[:, ih, :, :],
                        start=True, stop=True, skip_group_check=True,
                        perf_mode=mybir.MatmulPerfMode.DoubleRow,
                    )
                    nc.vector.tensor_scalar(
                        out=ysb[:, yslot, h:h + 1, :], in0=pt[:],
                        scalar1=float(K8), scalar2=float(B8),
                        op0=mybir.AluOpType.mult, op1=mybir.AluOpType.add,
                    )
                y8 = ysb[:, yslot, :, :].bitcast(F8E5)
                nc.tensor.matmul(
                    qd_ps[:],
                    lhsT=ones_t.ap(), rhs=y8,
                    start=first, stop=last, skip_group_check=True,
                    perf_mode=mybir.MatmulPerfMode.DoubleRow,
                )

            # pre-fence: chunk-0 ACT units (DMA group 0) interleaved with the
            # first epoch-0 quarters (pairs 0-7: slices 8-11 = group 2, own
            # i-lo rhs = slice 0 = group 0)
            pre = [("act", r, 0) for r in range(8)]
            preq = [("q", p, 0) for p in range(4, 12)]
            order = [pre[0], pre[1], preq[0], preq[1], pre[2], preq[2],
                     preq[3], pre[3], preq[4], preq[5], pre[4], preq[6],
                     preq[7], pre[5], pre[6], pre[7]]
            for it in order:
                if it[0] == "act":
                    act_unit(it[1], it[2])
                else:
                    dve_quarter(it[1], it[2])

            # fence: standalone LDWEIGHTS per not-yet-consumed DMA group; a
            # pure weight load has no psum operand, so it carries exactly ONE
            # wait (that group's DMA semaphore). Later PE instructions then
            # transitively dominate all input DMAs (1-wait walrus limit).
            for s in (7,):      # group 1 (groups 0/2 are consumed pre-fence)
                nc.tensor.ldweights(
                    weights=xin[:, s, :, 0:128],
                    perf_mode=mybir.MatmulPerfMode.DoubleRow,
                )

            # main: remaining ACT units (c=1..3 all blocks, c=4 only i-hi
            # blocks) interleaved with the remaining quarters (epoch-0 pairs
            # 8-15 incl slice-15 i-lo pairs 14/15, then epoch-1 pairs 0-13;
            # the QD bank is evacuated between epochs and reused)
            units = ([(r, 1) for r in range(8)] + ["colsum"]
                     + [(r, 2) for r in range(8)] + ["colsum2"])
            quarters = ([(p, 0) for p in range(0, 4)]
                        + [None]
                        + [(p, 1) for p in range(NPAIR)])

            def colsum_section(yt, sb0, sb1, dram_out):
                # QC [128,512]: two epochs (one per 512-chunk of the SYM
                # window); 4 pair DR-ones-mms accumulate, then evac + store
                qc_ps = act_pool.tile([128, 1024], F32, tag="act")
                for chunk, (sbuf, dram) in enumerate(
                        [(sb0, dram_out[:, 0:512]),
                         (sb1, dram_out[:, 512:1024])]):
                    for pair in range(4):
                        y8s = yt.ap()[
                            :, pair, :, 512 * chunk:512 * (chunk + 1)
                        ].bitcast(F8E5)
                        nc.tensor.matmul(
                            qc_ps[:, 512 * chunk:512 * (chunk + 1)],
                            lhsT=ones_t.ap(), rhs=y8s,
                            start=(pair == 0), stop=(pair == 3),
                            skip_group_check=True,
                            perf_mode=mybir.MatmulPerfMode.DoubleRow,
                        )
                    nc.scalar.activation(
                        out=sbuf.ap(), in_=qc_ps[0:1, 512 * chunk:512 * (chunk + 1)],
                        func=mybir.ActivationFunctionType.Copy, bias=0.0,
                    )
                    nc.gpsimd.dma_start(out=dram, in_=sbuf.ap())
            def emit_q():
                q = quarters.pop(0)
                if q is None:
                    # epoch-0 QD evac (DVE) + store; epoch-1 reuses the bank
                    nc.vector.tensor_copy(qd_sb0_t.ap(), qd_ps[0:1, :])
                    nc.gpsimd.dma_start(out=qd_out[:, 0:512],
                                        in_=qd_sb0_t.ap())
                else:
                    dve_quarter(*q)

            for _ in range(2):
                if quarters:
                    emit_q()
            for u in units:
                if u == "colsum":
                    colsum_section(ysym_t, qc_sb0_t, qc_sb1_t, qc_out)
                    continue
                if u == "colsum2":
                    colsum_section(ysym2_t, qc2_sb0_t, qc2_sb1_t, qc2_out)
                    continue
                act_unit(*u)
                if quarters:
                    emit_q()
            while quarters:
                emit_q()

            # epoch-1 QD evac; outputs split so each store DMA has one wait
            nc.vector.tensor_copy(qd_sb1_t.ap(), qd_ps[0:1, :])
            nc.gpsimd.dma_start(out=stats, in_=stats_t.ap())
            nc.gpsimd.dma_start(out=qd_out[:, 512:1024],
                                in_=qd_sb1_t.ap())
    return nc


_NC_CACHE = None


def _get_nc():
    global _NC_CACHE
    if _NC_CACHE is None:
        _NC_CACHE = _build()
    return _NC_CACHE


def kernel(labels, all_features, all_features_cr, _trace=False):
    labels = np.asarray(labels)
    f = np.asarray(all_features, dtype=np.float32)
    f_cr = np.asarray(all_features_cr, dtype=np.float32)

    X = np.concatenate([f, f_cr], axis=0)                 # [M, D] f32
    X8 = (X * 16.0).astype(ml_dtypes.float8_e4m3)         # device quantization
    XT8 = np.ascontiguousarray(X8.T)                      # [D, M]

    in_maps = []
    for c in range(NCORES):
        xe = np.empty((4, 128, 4096), dtype=ml_dtypes.float8_e4m3)
        for s in range(NSLICE):
            gs = (2 * c + s) % NSLICE
            g, o = s // 4, (s % 4) * 1024
            xe[g, :, o:o + 512] = XT8[0:128, 512 * gs:512 * (gs + 1)]
            xe[g, :, o + 512:o + 1024] = XT8[128:256, 512 * gs:512 * (gs + 1)]
        in_maps.append({"xe": xe})

    nc = _get_nc()
    res = run_bass_kernel_spmd(
        nc, in_maps, core_ids=list(range(NCORES)), trace=_trace
    )
    kernel.last_exec_time_ns = res.exec_time_ns
    kernel.last_trace = res.instructions_and_trace
    kernel.last_results = res.results

    # ---- host epilogue (float64, O(M*D)) ----
    X8f = X8.astype(np.float64) / 16.0                    # device-seen features
    d_hat = np.sum(X8f * X8f, axis=1)                     # device diag of A

    Q1 = np.empty(M, dtype=np.float64)                    # c=1 units, incl diag
    e_shift = C_CAL * np.exp((C_D - 1.0) / T)
    e_shift_sym = np.exp((C_D - 1.0) / T)
    for core in range(NCORES):
        st = res.results[core]["stats"].astype(np.float64)    # [128, 32]
        qd = res.results[core]["qd"].astype(np.float64)[0]    # [1024]
        g0 = core * ROWS_PER_CORE
        for r in range(8):
            cols = st[:, 5 * r:5 * r + 4]
            acc = cols[:, 0]                                 # c_A=1 unit (c0)
            acc_sym = (cols[:, 1] + cols[:, 2]) * e_shift_sym  # SYM units
            i0 = g0 + 128 * r
            Q1[i0:i0 + 128] = (acc + acc_sym
                               + e_shift * qd[128 * r:128 * (r + 1)])

    # mirror contributions: core c's SYM colsums are the (dropped) slices
    # 14-15 sums for core (c+1)'s rows
    for core in range(NCORES):
        qc = res.results[core]["qc"].astype(np.float64)[0]    # [1024]
        qc2 = res.results[core]["qc2"].astype(np.float64)[0]
        g1 = ((core + 1) % NCORES) * ROWS_PER_CORE
        g2 = ((core + 2) % NCORES) * ROWS_PER_CORE
        Q1[g1:g1 + 1024] += C_SYM * e_shift_sym * qc
        Q1[g2:g2 + 1024] += C_SYM * e_shift_sym * qc2

    # diagonal 128-blocks, exactly in f64 (the device skips them)
    for b in range(M // 128):
        blk = X8f[128 * b:128 * (b + 1)]
        Sb = blk @ blk.T
        Q1[128 * b:128 * (b + 1)] += np.exp((Sb - 1.0) / T).sum(axis=1)

    # diag term recenters to exactly 1 (matching the reference's exp(0));
    # C_Q removes the systematic e4m3-quantization bias of the off-diag sum
    row_sum = 1.0 + C_Q * (Q1 * np.exp((1.0 - d_hat) / T) - 1.0)
    row_logsum = np.log(row_sum)

    Xd = X.astype(np.float64)
    lab = np.asarray(labels)
    all_labels = np.concatenate([lab, lab]).astype(np.float64)
    pos_f = (all_labels == 1).astype(np.float64)
    neg_f = 1.0 - pos_f
    P = pos_f.sum()
    U = neg_f.sum()

    d_true = np.sum(Xd * Xd, axis=1)
    w_pos = pos_f @ Xd
    pos_dot_raw = Xd @ w_pos
    spos = (pos_dot_raw - P * d_true) / T
    sup_row = spos - M * row_logsum
    loss_sup = np.sum(pos_f * (-sup_row / P)) / P

    partner = np.sum(Xd * np.roll(Xd, -N, axis=0), axis=1)
    unsup_row = (partner - d_true) / T - M * row_logsum
    loss_unsup = np.sum(neg_f * (-unsup_row / U)) / U

    return (np.float32(loss_sup), np.float32(loss_unsup))
